# revision 12
# baseline (speedup 1.0000x reference)
"""GQA attention forward (B=4, T=1024, D=2048, 32 q-heads / 8 kv-heads, RoPE,
causal) distributed over 8 TRN2 NeuronCores.

Sharding: head-parallel tensor parallelism. Core c owns q-heads 4c..4c+3 and
kv-head c (wq/wk/wv column shards). Attention output (sharded by head,
transposed layout [head_dim, tokens]) is re-sharded to token-parallel via
per-chunk AllToAlls (2 x 256 KB per batch, bf16); each core then computes its
token slice of the output projection against the full wo.

Token ownership (64-token interleave so every chunk-level A2A feeds every
core): core c owns, per batch b, tokens [b*1024 + c*64, +64) from chunk 0 and
[b*1024 + 512 + c*64, +64) from chunk 1. The two 64-token halves pair into an
M=128 lhsT for the wo matmuls. wo work for batch b-1 is emitted interleaved
into batch b's attention phase (which is ACT/exp-bound) to keep the PE dense
and the HAM clock-gate warm.

Device layouts (per core):
  xT   [2048, 4096]  bf16  - x transposed, tokens batch-major (HBM)
  qTb  [128, 2, 1024] bf16 - per batch, 2 heads/tile, RoPE'd, de-interleaved
  kTb  [128, 1024]   bf16  - kv-head kT duplicated in both 64-partition halves
  v_b  [128, 8, 80]  bf16  - PE-transposed per 128-token tile with a ones
                             column (softmax denominator trick)
  scores sT[k, q] in PSUM -> exp on ACT (scale=1/8 folded) -> bf16
  attn@v: lhsT = v_aug [128, 65], rhs = expT -> psum [65, 512] accumulated
  denominator: ACT copy row64 -> reciprocal_approx_fast -> gpsimd
  partition_broadcast -> one fused DVE mul (divide + f32->bf16 + copy to ao)
  A2A per (chunk, hp): in [8, 128, 64] -> out gathers core s's rows for my
  64-token slot; wo: out[t, e] = sum_c aog[c, t] * wo[c, e]

RoPE with de-interleaved head dims ([32 reals; 32 imags] per 64-row head):
  out = x*C + shift32(x*S), C = [c;c;...], S = [s;-s;s;-s] (host-built tiles).
"""

import sys

if "/opt/trn_rl_repo" not in sys.path:
    sys.path.insert(0, "/opt/trn_rl_repo")

import numpy as np
import ml_dtypes

import concourse.bass as bass
import concourse.mybir as mybir
import concourse.tile as tile
from concourse import bacc
from concourse.bass_utils import run_bass_kernel_spmd
from concourse.masks import make_identity, make_upper_triangular

BF16 = mybir.dt.bfloat16
F32 = mybir.dt.float32

B, T, D = 4, 1024, 2048
QH, KVH, HD = 32, 8, 64
N_CORES = 8
NT = B * T            # 4096 global tokens
NKO = D // 128        # 16 contraction subtiles
ROWS = NT // N_CORES  # 512 output rows per core
HPC = QH // N_CORES   # 4 q heads per core

_CACHE = {}


def _enable_ldw_opt():
    # walrus ships with --enable-ldw-opt=false hardcoded; LDWEIGHTS then
    # serializes with matmul streams (~+40% per matmul). Flip it on.
    import os
    if not os.environ.get("FORCE_LDW_ON"):
        return
    import concourse.bass_utils as _bu
    if getattr(_bu, "_ldw_patched", False):
        return
    _orig = _bu.run_command

    def _patched(argv, **kw):
        argv = [a.replace("--enable-ldw-opt=false", "--enable-ldw-opt=true")
                if isinstance(a, str) else a for a in argv]
        return _orig(argv, **kw)

    _bu.run_command = _patched
    _bu._ldw_patched = True


def _build():
    _enable_ldw_opt()
    nc = bacc.Bacc("TRN2", target_bir_lowering=False, debug=False,
                   num_devices=N_CORES)

    xT = nc.dram_tensor("xT", [8, 128, NKO, 512], BF16, kind="ExternalInput")
    wq = nc.dram_tensor("wq", [128, NKO, HPC * HD], BF16,
                        kind="ExternalInput")
    wkv = nc.dram_tensor("wkv", [128, NKO, 2 * HD], BF16,
                         kind="ExternalInput")
    wo = nc.dram_tensor("wo", [128, NKO, D], BF16, kind="ExternalInput")
    ct = nc.dram_tensor("ctile", [128, T], BF16, kind="ExternalInput")
    st = nc.dram_tensor("stile", [128, T], BF16, kind="ExternalInput")
    out = nc.dram_tensor("out", [ROWS, D], F32, kind="ExternalOutput")

    xT_r = xT.ap()
    wq_r = wq.ap()
    wkv_r = wkv.ap()
    wo_r = wo.ap()

    import contextlib
    with tile.TileContext(nc) as tc, contextlib.ExitStack() as ctx:
        const = ctx.enter_context(tc.tile_pool(name="const", bufs=1))
        xp = ctx.enter_context(tc.tile_pool(name="xp", bufs=3))
        qkp = ctx.enter_context(tc.tile_pool(name="qkp", bufs=2))
        vp = ctx.enter_context(tc.tile_pool(name="vp", bufs=2))
        ep = ctx.enter_context(tc.tile_pool(name="ep", bufs=3))
        xsp = ctx.enter_context(tc.tile_pool(name="xsp", bufs=2))
        dnp = ctx.enter_context(tc.tile_pool(name="dnp", bufs=4))
        bcp = ctx.enter_context(tc.tile_pool(name="bcp", bufs=4))
        aop = ctx.enter_context(tc.tile_pool(name="aop", bufs=2))
        gp = ctx.enter_context(tc.tile_pool(name="gp", bufs=2))
        op = ctx.enter_context(tc.tile_pool(name="op", bufs=2))
        dram = ctx.enter_context(tc.tile_pool(name="dram", bufs=1,
                                              space="DRAM"))
        pp = ctx.enter_context(tc.tile_pool(name="pp", bufs=1, space="PSUM"))
        sp = ctx.enter_context(tc.tile_pool(name="sp", bufs=2, space="PSUM"))
        ap = ctx.enter_context(tc.tile_pool(name="ap", bufs=2, space="PSUM"))

        # constants / weights
        wq_sb = const.tile([128, NKO, HPC * HD], BF16, tag="wq")
        nc.sync.dma_start(wq_sb[:], wq_r)
        wkv_sb = const.tile([128, NKO, 2 * HD], BF16, tag="wkv")
        ct_sb = const.tile([128, T], BF16, tag="ct")
        st_sb = const.tile([128, T], BF16, tag="st")

        def aux_load():
            nc.sync.dma_start(wkv_sb[:], wkv_r)
            nc.sync.dma_start(ct_sb[:], ct.ap())
            nc.sync.dma_start(st_sb[:], st.ap())
        utri = const.tile([128, 128], BF16, tag="utri")
        make_upper_triangular(nc, utri[:], val=1.0, diag=True)
        utri2 = const.tile([128, 2, 128], BF16, tag="utri2")
        nc.vector.tensor_copy(utri2[:, 0, :], utri[:])
        nc.vector.tensor_copy(utri2[:, 1, :], utri[:])
        ident = const.tile([64, 64], BF16, tag="ident")
        make_identity(nc, ident[:])

        # A2A staging: one in/out pair per (batch, chunk, hp); slot j
        # carries my 128 hp-half rows for 64-token slice j of the chunk
        in_h = [[dram.tile([N_CORES, 128, 64], BF16,
                           tag=f"a2a_in{k}_{hp}", name=f"a2a_in{k}_{hp}")
                 for hp in range(2)] for k in range(2 * B)]
        out_h = [[dram.tile([N_CORES, 128, 64], BF16,
                            tag=f"a2a_out{k}_{hp}", name=f"a2a_out{k}_{hp}")
                  for hp in range(2)] for k in range(2 * B)]

        # full wo stays resident; loaded on the sync queue behind batch-0's
        # x so it doesn't steal HBM bandwidth from the critical-path loads
        wo_sb = const.tile([128, NKO, D], BF16, tag="wo")

        def wo_load():
            for n in range(4):
                nc.sync.dma_start(wo_sb[:, :, n * 512:n * 512 + 512],
                                  wo_r[:, :, n * 512:n * 512 + 512])

        warm_in = dram.tile([N_CORES, 128, 64], BF16, tag="a2a_warm_in",
                            name="a2a_warm_in")
        warm_out = dram.tile([N_CORES, 128, 64], BF16, tag="a2a_warm_out",
                             name="a2a_warm_out")
        nc.gpsimd.collective_compute(
            "AllToAll", mybir.AluOpType.bypass,
            replica_groups=[list(range(N_CORES))],
            ins=[warm_in.opt()], outs=[warm_out.opt()])

        def wo_gather(bb):
            # gather the per-chunk A2A outputs of batch bb into
            # aog [128, hp, s, 128]; token cols 0:64 = chunk 0, 64:128 = ch 1
            aog = gp.tile([128, 2, N_CORES, 128], BF16, tag="aog")
            for half in range(2):
                for hp in range(2):
                    nc.gpsimd.dma_start(
                        aog[:, hp, :, half * 64:half * 64 + 64],
                        out_h[2 * bb + half][hp].rearrange(
                            "s p q -> p s q"))
            return aog

        def wo_n(bb, aog, n):
            # one 512-wide column block of my 128 tokens x wo
            ps = pp.tile([128, 512], F32, tag="mm", bufs=2)
            for kk in range(2 * N_CORES):
                s, hp = kk // 2, kk % 2
                nc.tensor.matmul(
                    ps[:], aog[:, hp, s, :],
                    wo_sb[:, kk, n * 512:n * 512 + 512],
                    start=(kk == 0), stop=(kk == 2 * N_CORES - 1))
            ot = op.tile([128, 512], F32, tag="ot", bufs=2)
            nc.vector.tensor_copy(ot[:], ps[:])
            nc.scalar.dma_start(
                out.ap()[bb * 128:bb * 128 + 128,
                         n * 512:n * 512 + 512], ot[:])

        def rope(dst, xs, xs2, ps, rows, cs_sl, ss_sl):
            # dst = ps*C + shift32(ps*S) over `rows` partitions (64 or 128)
            nc.vector.scalar_tensor_tensor(
                dst, ps[0:rows], 1.0, cs_sl[0:rows],
                mybir.AluOpType.mult, mybir.AluOpType.mult)
            nc.vector.scalar_tensor_tensor(
                xs[0:rows], ps[0:rows], 1.0, ss_sl[0:rows],
                mybir.AluOpType.mult, mybir.AluOpType.mult)
            # shift-by-32 within each 64-row half (cross-partition copies)
            for g in range(rows // 32):
                a, b_ = g * 32, (g ^ 1) * 32
                nc.vector.tensor_copy(xs2[a:a + 32], xs[b_:b_ + 32])
            nc.vector.tensor_add(dst, dst, xs2[0:rows])

        for b in range(B):
            aog_prev = [None]  # gathered lazily at first wo_n use

            xc = []
            for half in range(2):
                xt = xp.tile([128, NKO, 512], BF16, tag="x")
                for kq in range(4):
                    nc.sync.dma_start(
                        xt[:, kq * 4:kq * 4 + 4, :],
                        xT_r[b * 2 + half, :, kq * 4:kq * 4 + 4])
                xc.append(xt)
            if b == 0:
                aux_load()
                wo_load()

            qTb = qkp.tile([128, 2, 1024], BF16, tag="qT")
            kTb = qkp.tile([128, 1024], BF16, tag="kT")

            def q_proj(hp, half):
                t0 = half * 512
                ps = pp.tile([128, 512], F32, tag="mm", bufs=2)
                for ko in range(NKO):
                    nc.tensor.matmul(
                        ps[:], wq_sb[:, ko, hp * 128:hp * 128 + 128],
                        xc[half][:, ko, :],
                        start=(ko == 0), stop=(ko == NKO - 1))
                xs = xsp.tile([128, 512], BF16, tag="xs")
                xs2 = xsp.tile([128, 512], BF16, tag="xs2")
                csl = ct_sb[:, t0:t0 + 512]
                ssl = st_sb[:, t0:t0 + 512]
                rope(qTb[:, hp, t0:t0 + 512], xs, xs2, ps, 128, csl, ssl)

            # q heads hp=0 first, then kv (so attention hp=0 can start),
            # then q heads hp=1
            for half in range(2):
                q_proj(0, half)

            vstages = []
            for half in range(2):
                t0 = half * 512
                ps = pp.tile([128, 512], F32, tag="mm", bufs=2)
                for ko in range(NKO):
                    nc.tensor.matmul(
                        ps[:], wkv_sb[:, ko, :], xc[half][:, ko, :],
                        start=(ko == 0), stop=(ko == NKO - 1))
                xs = xsp.tile([128, 512], BF16, tag="xs")
                xs2 = xsp.tile([128, 512], BF16, tag="xs2")
                csl = ct_sb[:, t0:t0 + 512]
                ssl = st_sb[:, t0:t0 + 512]
                rope(kTb[0:64, t0:t0 + 512], xs, xs2, ps, 64, csl, ssl)
                nc.vector.tensor_copy(kTb[64:128, t0:t0 + 512],
                                      kTb[0:64, t0:t0 + 512])
                vstage = xsp.tile([64, 512], BF16, tag="vstage",
                                  name=f"vstage{half}")
                nc.scalar.copy(vstage[:], ps[64:128, :])
                vstages.append(vstage)

            # v into natural layout [tok, 64] + ones column (PE transpose)
            v_b = vp.tile([128, 8, 80], BF16, tag="v")
            for j in range(8):
                tps = pp.tile([128, 64], BF16, tag="mm", bufs=2)
                nc.tensor.transpose(
                    tps[:],
                    vstages[j // 4][:, (j % 4) * 128:(j % 4) * 128 + 128],
                    ident[:])
                nc.scalar.copy(v_b[:, j, 0:HD], tps[:])
            nc.any.memset(v_b[:, :, HD:80], 1.0)

            for half in range(2):
                q_proj(1, half)

            # attention: all 4 heads advance together per (chunk, k-tile) so
            # the PE stream stays dense; K=64 score matmuls for h01=0/1 sit
            # in disjoint row groups and overlap in the array
            for c in range(2):
                ao = aop.tile([128, 2, 512], BF16, tag="ao")
                for hp in range(2):
                    po2 = [ap.tile([65, 512], F32, tag="attn",
                                   name=f"po{h01}") for h01 in range(2)]
                    nj = 4 * c + 4
                    for j in range(nj):
                        q_lo = max(c * 512, j * 128)
                        N = (c + 1) * 512 - q_lo
                        q_rel = q_lo - c * 512
                        # both heads' scores into one 2-bank psum tile so
                        # exp and mask run once over [128, 2, N]
                        ps = sp.tile([128, 2, 512], F32, tag="score")
                        et = ep.tile([128, 2, 512], BF16, tag="et")
                        for h01 in range(2):
                            hbase = h01 * 64
                            nc.tensor.matmul(
                                ps[:, h01, :N],
                                kTb[hbase:hbase + 64,
                                    j * 128:j * 128 + 128],
                                qTb[hbase:hbase + 64, hp, q_lo:q_lo + N],
                                start=True, stop=True)
                        nc.scalar.activation(
                            et[:, :, :N], ps[:, :, :N],
                            mybir.ActivationFunctionType.Exp, scale=0.125)
                        if j >= 4 * c:
                            nc.vector.tensor_mul(
                                et[:, :, 0:128], et[:, :, 0:128], utri2[:])
                        for h01 in range(2):
                            nc.tensor.matmul(
                                po2[h01][:, q_rel:512], v_b[:, j, :65],
                                et[:, h01, :N],
                                start=(j == 0), stop=(j == nj - 1))
                    # softmax normalization, fused with the PSUM->SBUF copy:
                    # 1/den via fast reciprocal, partition-broadcast, one mul
                    for h01 in range(2):
                        po = po2[h01]
                        dn = dnp.tile([1, 512], F32, tag="dn")
                        nc.vector.tensor_copy(dn[:], po[64:65, :])
                        rec = dnp.tile([1, 512], F32, tag="rec")
                        nc.vector.reciprocal_approx_fast(rec[:], dn[:])
                        bc = bcp.tile([64, 512], F32, tag="bc")
                        nc.gpsimd.partition_broadcast(bc[:], rec[:])
                        nc.vector.tensor_mul(
                            ao[h01 * 64:h01 * 64 + 64, hp, :],
                            po[0:64, :], bc[:])
                    # re-shard this (chunk, hp): slot j carries my rows for
                    # 64-token slice j of the chunk
                    nc.sync.dma_start(
                        in_h[2 * b + c][hp].rearrange("j p q -> p j q"),
                        ao[:, hp, :])
                    nc.gpsimd.collective_compute(
                        "AllToAll", mybir.AluOpType.bypass,
                        replica_groups=[list(range(N_CORES))],
                        ins=[in_h[2 * b + c][hp].opt()],
                        outs=[out_h[2 * b + c][hp].opt()])
                    # interleave prev batch's wo into the exp-bound phase;
                    # batch 0's A2As absorb residual start skew, so batch 1
                    # consumes them only in its second chunk
                    if b >= 1:
                        for n in ([2 * c + hp] if b != 1 else
                                  ([2 * hp, 2 * hp + 1] if c == 1 else [])):
                            if aog_prev[0] is None:
                                aog_prev[0] = wo_gather(b - 1)
                            wo_n(b - 1, aog_prev[0], n)

        aog_last = wo_gather(B - 1)
        for n in range(4):
            wo_n(B - 1, aog_last, n)

    nc.compile()
    return nc


def _tile_k(w):
    # [D, M] -> [128, D//128, M] with d = ko*128 + p, contiguous per partition
    return np.ascontiguousarray(
        w.reshape(NKO, 128, w.shape[1]).transpose(1, 0, 2))


def _prep_inputs(x, wq, wk, wv, wo, cos, sin):
    bf = ml_dtypes.bfloat16
    x2 = x.reshape(NT, D).T  # [D, NT]
    # [8 chunks, 128, NKO, 512]: one contiguous 16KB run per partition
    xt = np.ascontiguousarray(
        x2.reshape(NKO, 128, 8, 512).transpose(2, 1, 0, 3)).astype(bf)
    # de-interleave rope pairs within each head: col j -> (j%2)*32 + j//2
    wq_p = wq.reshape(D, QH, 32, 2).transpose(0, 1, 3, 2).reshape(D, QH * HD)
    wk_p = wk.reshape(D, KVH, 32, 2).transpose(0, 1, 3, 2).reshape(D, KVH * HD)
    cosT = np.ascontiguousarray(cos.T)  # [32, T]
    sinT = np.ascontiguousarray(sin.T)
    ctile = np.concatenate([cosT] * 4, axis=0).astype(bf)
    stile = np.concatenate([sinT, -sinT, sinT, -sinT], axis=0).astype(bf)
    wo_t = _tile_k(wo).astype(bf)
    in_maps = []
    for c in range(N_CORES):
        wq_s = _tile_k(wq_p[:, c * 256:(c + 1) * 256]).astype(bf)
        wkv_s = _tile_k(np.concatenate(
            [wk_p[:, c * 64:(c + 1) * 64], wv[:, c * 64:(c + 1) * 64]],
            axis=1)).astype(bf)
        in_maps.append({
            "xT": xt, "wq": wq_s, "wkv": wkv_s, "wo": wo_t,
            "ctile": ctile, "stile": stile,
        })
    return in_maps


def _run(inputs, trace=False):
    if "nc" not in _CACHE:
        _CACHE["nc"] = _build()
    nc = _CACHE["nc"]
    in_maps = _prep_inputs(
        np.asarray(inputs["x"], dtype=np.float32),
        np.asarray(inputs["wq"], dtype=np.float32),
        np.asarray(inputs["wk"], dtype=np.float32),
        np.asarray(inputs["wv"], dtype=np.float32),
        np.asarray(inputs["wo"], dtype=np.float32),
        np.asarray(inputs["cos"], dtype=np.float32),
        np.asarray(inputs["sin"], dtype=np.float32),
    )
    res = run_bass_kernel_spmd(nc, in_maps, core_ids=list(range(N_CORES)),
                               trace=trace)
    full = np.empty((NT, D), dtype=np.float32)
    for c in range(N_CORES):
        o = res.results[c]["out"]
        for b in range(B):
            g0 = b * T + c * 64
            g1 = b * T + 512 + c * 64
            full[g0:g0 + 64] = o[b * 128:b * 128 + 64]
            full[g1:g1 + 64] = o[b * 128 + 64:b * 128 + 128]
    return full.reshape(B, T, D), res


def kernel(**inputs) -> np.ndarray:
    out, _ = _run(inputs, trace=False)
    return out


def kernel_traced(**inputs):
    out, res = _run(inputs, trace=True)
    return out, res
